# revision 1
# baseline (speedup 1.0000x reference)
"""GroupedQueryAttention on 8 Trainium2 NeuronCores.

Tensor-parallel over heads (per sharding_hint): each of the 8 cores owns 2 of
the 16 q-heads (Wq output columns + Wo input rows sharded). KV projections are
small ([2048x512]) and replicated; each core slices out the one KV group its
heads need. Partial out-projections are summed with an all-reduce (psum).
"""
import numpy as np
import jax
import jax.numpy as jnp
from jax.sharding import Mesh, PartitionSpec as P
from jax.experimental.shard_map import shard_map
from functools import partial

B, S, D_IN = 2, 2048, 2048
H, G, D = 16, 4, 128
NC = 8
HPC = H // NC          # heads per core
EPS = 1e-6

_cached = {}


def _rms_norm(x, w):
    xf = x.astype(jnp.float32)
    var = jnp.mean(xf * xf, axis=-1, keepdims=True)
    return (xf * jax.lax.rsqrt(var + EPS) * w).astype(x.dtype)


def _rope(x, cos, sin):
    half = x.shape[-1] // 2
    x1, x2 = x[..., :half], x[..., half:]
    rotated = jnp.concatenate([-x2, x1], axis=-1)
    return x * cos[None, None] + rotated * sin[None, None]


def _shard_body(x, mask, cos, sin, wq_l, wk, wv, wo_l, qw, kw):
    # wq_l: [D_IN, HPC*D] local q-head columns; wo_l: [HPC*D, D_IN] local rows
    b, s, _ = x.shape
    scaling = D ** -0.5
    q = (x @ wq_l).reshape(b, s, HPC, D).transpose(0, 2, 1, 3)   # [b,hpc,s,D]
    k = (x @ wk).reshape(b, s, G, D).transpose(0, 2, 1, 3)       # [b,G,s,D]
    v = (x @ wv).reshape(b, s, G, D).transpose(0, 2, 1, 3)
    # this core's heads are global heads [HPC*idx, HPC*idx+HPC) -> one group
    idx = jax.lax.axis_index("tp")
    g = (idx * HPC) // (H // G)
    k = jax.lax.dynamic_slice_in_dim(k, g, 1, axis=1)            # [b,1,s,D]
    v = jax.lax.dynamic_slice_in_dim(v, g, 1, axis=1)
    q = _rms_norm(q, qw)
    k = _rms_norm(k, kw)
    q = _rope(q, cos, sin)
    k = _rope(k, cos, sin)
    k = jnp.broadcast_to(k, (b, HPC, s, D))
    v = jnp.broadcast_to(v, (b, HPC, s, D))
    scores = jnp.einsum("bhqd,bhkd->bhqk", q * scaling, k)
    scores = jnp.where(mask[None, None], -jnp.inf, scores)
    attn = jax.nn.softmax(scores.astype(jnp.float32), axis=-1).astype(q.dtype)
    ctx = jnp.einsum("bhqk,bhkd->bhqd", attn, v)
    ctx = ctx.transpose(0, 2, 1, 3).reshape(b, s, HPC * D)
    part = ctx @ wo_l                                            # [b,s,D_IN]
    return jax.lax.psum(part, "tp")


def _build():
    devs = jax.devices()[:NC]
    mesh = Mesh(np.asarray(devs), ("tp",))
    spec_r = P()
    fn = shard_map(
        _shard_body,
        mesh=mesh,
        in_specs=(spec_r, spec_r, spec_r, spec_r,
                  P(None, "tp"),      # wq [D_IN, H*D] cols sharded by head
                  spec_r, spec_r,
                  P("tp", None),      # wo [H*D, D_IN] rows sharded by head
                  spec_r, spec_r),
        out_specs=spec_r,
        check_rep=False,
    )
    return jax.jit(fn)


def kernel(x, mask, cos, sin, Wq, Wk, Wv, Wo, q_norm_w, k_norm_w):
    if "fn" not in _cached:
        _cached["fn"] = _build()
    fn = _cached["fn"]
    out = fn(
        jnp.asarray(x), jnp.asarray(mask), jnp.asarray(cos), jnp.asarray(sin),
        jnp.asarray(Wq), jnp.asarray(Wk), jnp.asarray(Wv), jnp.asarray(Wo),
        jnp.asarray(q_norm_w), jnp.asarray(k_norm_w),
    )
    return np.asarray(jax.block_until_ready(out))

